# revision 33
# baseline (speedup 1.0000x reference)
"""Self-contained Trainium2 Bass kernel for gated attention (sparse_attention).

Reference computation (per batch b):
    q = split_heads(x @ Wq) * DH**-0.5        # (H, n, DH)
    k, v = split_heads(x @ Wkv)               # (H, n, DH) each
    dots = q k^T + attn_bias ; masked softmax over j
    out = (attn @ v) reshaped to (n, H*DH)
    out = out * sigmoid(x @ Wg + bg)
    return out @ Wo + bo

Sharding: 8 cores = 4 batches x 2 query-row halves, physical j
coordinates.  Each core projects k/v only for its OWN 512 rows; the
halves are exchanged between pair cores with a 2-rank AllGather
(DRAM bounce), so k/v work is not duplicated.  Per-core outputs are
disjoint.

v3 design: one fully-pipelined TileContext (no phase barriers).
  - x input is just the core's own 512 rows; x^T built with 32 PE
    transposes.
  - bias (host-side exp'ed, bf16) streams from t=0 while projections
    and the k/v exchange run.
  - attention per head-pair dt: QK pair (row-groups) and AV pair
    (col-groups) run as concurrent matmul pairs; row sums are two
    concurrent M=1 matmuls into one PSUM bank; 1/sum comes from
    exp(-ln(s)) on ACT applied to the PE-broadcast tile.
  - engine split: ACT = exp + sigmoid + recip; DVE = PSUM drains,
    gating, output adds, ~60% of bias mults; GPSIMD = the rest.
"""
import sys
import types

import numpy as np
import ml_dtypes

# ---------------------------------------------------------------------------
# Environment shims (axon container): NTFF profile hook + walrus drain fix.
# ---------------------------------------------------------------------------


def _install_axon_ntff_hook():
    try:
        import antenv
    except ImportError:
        return
    if hasattr(antenv, "axon_hooks"):
        return
    mod = types.ModuleType("antenv.axon_hooks")
    mod._hook = None

    def set_axon_ntff_profile_hook(h):
        mod._hook = h

    def get_axon_ntff_profile_hook():
        return mod._hook

    mod.set_axon_ntff_profile_hook = set_axon_ntff_profile_hook
    mod.get_axon_ntff_profile_hook = get_axon_ntff_profile_hook
    sys.modules["antenv.axon_hooks"] = mod
    antenv.axon_hooks = mod
    try:
        from trn_agent_boot.trn_boot import _ntff_profile_via_ctypes

        hook = _ntff_profile_via_ctypes("/opt/axon/libaxon_pjrt.so")
        if hook is not None:
            set_axon_ntff_profile_hook(hook)
    except Exception:
        pass


_install_axon_ntff_hook()

import concourse.bass as bass  # noqa: E402
import concourse.tile as tile  # noqa: E402
import concourse.mybir as mybir  # noqa: E402
from concourse.bass_utils import run_bass_kernel_spmd  # noqa: E402
from concourse.masks import make_identity  # noqa: E402
from concourse.tile import ScopedClock  # noqa: E402


def _patch_tile_drain():
    """The installed walrus accepts only one sync-wait per Drain; Tile's
    tail drain carries one wait per outstanding semaphore.  Split them
    across a chain of single-wait drains (same engine => same semantics)."""

    def _drain_and_barrier(self, tick_clock, wait_clock):
        nc = self.nc
        drain_inst = nc.sync.drain()
        wait_clock.add_sem_waits(
            drain_inst.ins, ScopedClock({None: tick_clock.global_clock})
        )
        si = drain_inst.ins.sync_info
        if si is not None and len(si.on_wait) > 1:
            waits = list(si.on_wait)
            drain_inst.ins.sync_info = mybir.SyncInfo(
                on_wait=waits[:1], on_update=list(si.on_update)
            )
            for w in waits[1:]:
                extra = nc.sync.drain()
                extra.ins.sync_info = mybir.SyncInfo(on_wait=[w], on_update=[])

        nc.all_engine_barrier()
        assert self.sems is not None
        popped = nc._tile_sem_poison_stack.pop()
        assert popped is self._sem_poison
        nc.clear_and_free_semaphores(list(self.sems.allocated().values()))
        nc.all_engine_barrier()

    tile.TileContext._drain_and_barrier = _drain_and_barrier


_patch_tile_drain()


def _legalize_waits(nc, max_waits=1):
    """Walrus in this container accepts at most one sync-wait per lowered
    instruction.  Move surplus waits onto single-wait NoOps inserted just
    before the instruction on the same engine."""
    nid = 0
    n_split = 0
    for f in nc.m.functions:
        for bb in f.blocks:
            out = []
            changed = False
            for inst in bb.instructions:
                si = inst.sync_info
                if si is not None and len(si.on_wait) > max_waits:
                    waits = list(si.on_wait)
                    for w in waits[:-1]:
                        nop = mybir.InstNoOp(name=f"WSPLIT-{nid}")
                        nid += 1
                        nop.engine = inst.engine
                        nop.sync_info = mybir.SyncInfo(on_wait=[w], on_update=[])
                        out.append(nop)
                    inst.sync_info = mybir.SyncInfo(
                        on_wait=[waits[-1]], on_update=list(si.on_update)
                    )
                    changed = True
                    n_split += 1
                out.append(inst)
            if changed:
                bb.instructions = out
    return n_split


# ---------------------------------------------------------------------------
# Problem constants (hardcoded per spec).
# ---------------------------------------------------------------------------
B, N, D = 4, 1024, 1024
H, DH = 8, 64
INNER = H * DH  # 512
M = N // 2  # 512 query rows per core
N_CORES = 8
P = 128
F32 = mybir.dt.float32
BF16 = mybir.dt.bfloat16

CT = D // P  # 8 contraction tiles over feature dim
DT = INNER // P  # 4 head pairs
NT = N // P  # 8 tiles over keys j (full)
JO = M // P  # 4 own j tiles
IB = M // P  # 4 tiles over query rows
KVW = DT * M + JO * M  # 4096: packed exchange row width (k | v)

Exp = mybir.ActivationFunctionType.Exp
Ln = mybir.ActivationFunctionType.Ln
Sigmoid = mybir.ActivationFunctionType.Sigmoid
MUL = mybir.AluOpType.mult
ADD = mybir.AluOpType.add


def _build_graph():
    nc = bass.Bass()
    x_ext = nc.declare_dram_parameter("x", [M, D], BF16, isOutput=False)
    bias_ext = nc.declare_dram_parameter("bias", [DT, N, 2, M], BF16, isOutput=False)
    wq_ext = nc.declare_dram_parameter("wq", [D, INNER], BF16, isOutput=False)
    wkv_ext = nc.declare_dram_parameter("wkv", [D, 2 * INNER], BF16, isOutput=False)
    wg_ext = nc.declare_dram_parameter("wg", [D, INNER], BF16, isOutput=False)
    nbg_ext = nc.declare_dram_parameter("nbg", [P, DT], F32, isOutput=False)
    wo_ext = nc.declare_dram_parameter("wo", [INNER, D], BF16, isOutput=False)
    bob_ext = nc.declare_dram_parameter("bob", [P, D], F32, isOutput=False)
    out_ext = nc.declare_dram_parameter("out", [M, D], F32, isOutput=True)

    with tile.TileContext(nc) as tc:
        with (
            tc.tile_pool(name="persist", bufs=1) as persist,
            tc.tile_pool(name="rings", bufs=1) as rings,
            tc.tile_pool(name="dram", bufs=1, space="DRAM") as dram,
            tc.tile_pool(name="ps", bufs=1, space="PSUM") as ps,
        ):
            # ---------------- persistent SBUF ----------------
            x_sb = persist.tile([P, IB, D], BF16, name="x_sb")
            xT = persist.tile([P, CT, M], BF16, name="xT")  # own x^T [c, i]
            kT = persist.tile([P, DT, N], BF16, name="kT")  # k^T  [dI, j] full
            vA = persist.tile([P, DT * NT * 2, 64], BF16, name="vA")  # v slots
            kvx = persist.tile([P, KVW], BF16, name="kvx")  # exchange staging
            qT = persist.tile([P, DT, M], BF16, name="qT")
            gT = persist.tile([P, DT, M], BF16, name="gT")  # sigmoid gates
            gatedT = persist.tile([P, DT, M], BF16, name="gatedT")
            wq_sb = persist.tile([P, CT, INNER], BF16, name="wq_sb")
            wkv_sb = persist.tile([P, CT, 2 * INNER], BF16, name="wkv_sb")
            wg_sb = persist.tile([P, CT, INNER], BF16, name="wg_sb")
            wo_sb = persist.tile([P, DT, D], BF16, name="wo_sb")
            nbg_sb = persist.tile([P, DT], F32, name="nbg_sb")
            bob_sb = persist.tile([P, D], F32, name="bob_sb")
            srow = persist.tile([P, 2, M], BF16, name="srow")  # sums @p0/p32
            ones_sb = persist.tile([P, P], BF16, name="ones_sb")
            ident = persist.tile([P, P], BF16, name="ident")
            warm_src = persist.tile([P, 512], BF16, name="warm_src")

            bias_tiles = {
                dt: rings.tile([P, NT * 2 * M], BF16, tag="bias", bufs=3, name="bias_sb")
                for dt in range(DT)
            }

            # DRAM bounce buffers for the pairwise k/v AllGather
            bounce_in = dram.tile([P, KVW], BF16, name="bounce_in")
            bounce_out = dram.tile([2, P, KVW], BF16, name="bounce_out")

            # ---------------- DMA: sync ring ----------------
            nc.sync.dma_start(
                out=x_sb, in_=x_ext.rearrange("(nt p) d -> p nt d", p=P)
            )
            PAIRS = [[0, 1], [2, 3], [4, 5], [6, 7]]

            def emit_exchange():
                # Emitted AFTER the k/v projections (Tile deps are
                # program-order; an earlier read would see garbage).
                nc.sync.dma_start(out=bounce_in, in_=kvx)
                nc.gpsimd.collective_compute(
                    "AllGather",
                    mybir.AluOpType.bypass,
                    ins=[bounce_in.opt()],
                    outs=[bounce_out.opt()],
                    replica_groups=PAIRS,
                )
                for t in range(2):
                    nc.sync.dma_start(
                        out=kT[:, :, t * M : (t + 1) * M],
                        in_=bounce_out[t, :, 0 : DT * M].rearrange(
                            "p (dt i) -> p dt i", dt=DT
                        ),
                    )
                    # vA slot order is (jt_phys, dt, h): one contiguous DMA
                    nc.sync.dma_start(
                        out=vA[:, t * 32 : (t + 1) * 32, :],
                        in_=bounce_out[t, :, DT * M :],
                    )
                for dt in range(2, DT):
                    nc.sync.dma_start(
                        out=bias_tiles[dt].rearrange(
                            "p (jt h i) -> p jt h i", jt=NT, h=2
                        ),
                        in_=bias_ext[dt].rearrange("(jt p) h i -> p jt h i", p=P),
                    )

            # ---------------- DMA: scalar ring = weights, then bias 0/1 ----
            nc.scalar.dma_start(
                out=wkv_sb, in_=wkv_ext.rearrange("(ct p) i -> p ct i", p=P)
            )
            nc.scalar.dma_start(
                out=wq_sb, in_=wq_ext.rearrange("(ct p) i -> p ct i", p=P)
            )
            nc.scalar.dma_start(
                out=wg_sb, in_=wg_ext.rearrange("(ct p) i -> p ct i", p=P)
            )
            nc.scalar.dma_start(out=nbg_sb, in_=nbg_ext[:])
            for dt in range(2):
                nc.scalar.dma_start(
                    out=bias_tiles[dt].rearrange(
                        "p (jt h i) -> p jt h i", jt=NT, h=2
                    ),
                    in_=bias_ext[dt].rearrange("(jt p) h i -> p jt h i", p=P),
                )
            nc.scalar.dma_start(
                out=wo_sb, in_=wo_ext.rearrange("(dt p) i -> p dt i", p=P)
            )
            nc.scalar.dma_start(out=bob_sb, in_=bob_ext[:])

            # ---------------- init ----------------
            nc.vector.memset(warm_src, 1.0)
            nc.vector.memset(ones_sb, 1.0)
            make_identity(nc, ident)

            # ---------------- PE warmup ----------------
            for i in range(6):
                wt = ps.tile([P, 512], F32, tag="small", bufs=2, name="warm")
                nc.tensor.matmul(
                    wt, lhsT=warm_src[:, 0:P], rhs=warm_src, start=True, stop=True,
                    skip_group_check=True,
                )

            # ---------------- x^T via PE transposes ----------------
            for nt in range(IB):
                for ct in range(CT):
                    pt = ps.tile([P, P], BF16, tag="small", bufs=2, name="pt")
                    nc.tensor.transpose(
                        pt, x_sb[:, nt, ct * P : (ct + 1) * P], ident
                    )
                    nc.vector.tensor_copy(
                        out=xT[:, ct, nt * P : (nt + 1) * P], in_=pt
                    )

            # ---------------- helpers ----------------
            def emit_kown(dt):
                # k^T own half -> kvx[:, dt*M:(dt+1)*M]
                pt = ps.tile([P, M], F32, tag="small", bufs=2, name="pk")
                for ct in range(CT):
                    nc.tensor.matmul(
                        pt,
                        lhsT=wkv_sb[:, ct, dt * P : (dt + 1) * P],
                        rhs=xT[:, ct, :],
                        start=(ct == 0),
                        stop=(ct == CT - 1),
                    )
                nc.vector.tensor_copy(out=kvx[:, dt * M : (dt + 1) * M], in_=pt)

            def emit_vown(jt):
                # v rows for own j-tile jt (all heads) -> kvx v section
                pt = ps.tile([P, INNER], F32, tag="small", bufs=2, name="pv")
                for ct in range(CT):
                    nc.tensor.matmul(
                        pt,
                        lhsT=xT[:, ct, jt * P : (jt + 1) * P],
                        rhs=wkv_sb[:, ct, INNER:],
                        start=(ct == 0),
                        stop=(ct == CT - 1),
                    )
                nc.vector.tensor_copy(
                    out=kvx[:, DT * M + jt * INNER : DT * M + (jt + 1) * INNER],
                    in_=pt,
                )

            def emit_qT(dt):
                pt = ps.tile([P, M], F32, tag="small", bufs=2, name="pq")
                for ct in range(CT):
                    nc.tensor.matmul(
                        pt,
                        lhsT=wq_sb[:, ct, dt * P : (dt + 1) * P],
                        rhs=xT[:, ct, :],
                        start=(ct == 0),
                        stop=(ct == CT - 1),
                    )
                nc.vector.tensor_copy(out=qT[:, dt, :], in_=pt)

            def emit_g(dt):
                pt = ps.tile([P, M], F32, tag="small", bufs=2, name="pg")
                for ct in range(CT):
                    nc.tensor.matmul(
                        pt,
                        lhsT=wg_sb[:, ct, dt * P : (dt + 1) * P],
                        rhs=xT[:, ct, :],
                        start=(ct == 0),
                        stop=(ct == CT - 1),
                    )
                nc.scalar.activation(
                    out=gT[:, dt, :],
                    in_=pt,
                    func=Sigmoid,
                    scale=1.0,
                    bias=nbg_sb[:, dt : dt + 1],
                )

            # attention state
            aT_tiles = {}
            pav_tiles = {}
            sums_tiles = {}

            def emit_qk(dt, jt):
                pd = ps.tile([P, 2, M], F32, tag="pd", bufs=2, name="pd")
                for hi in range(2):
                    po = 64 * hi
                    nc.tensor.matmul(
                        pd[:, hi, :],
                        lhsT=kT[po : po + 64, dt, jt * P : (jt + 1) * P],
                        rhs=qT[po : po + 64, dt, :],
                        start=True,
                        stop=True,
                    )
                aT = aT_tiles[dt]
                dst = aT[:, jt * 2 * M : (jt + 1) * 2 * M]
                nc.scalar.activation(
                    out=dst, in_=pd.rearrange("p a b -> p (a b)"), func=Exp, scale=1.0
                )
                beng = nc.gpsimd if jt in (1, 3, 5) else nc.vector
                beng.tensor_tensor(
                    dst, dst, bias_tiles[dt][:, jt * 2 * M : (jt + 1) * 2 * M], MUL
                )

            def emit_av(dt, jt):
                # AV pair on disjoint col groups (concurrent), then the two
                # M=1 row-sum matmuls on groups 0/1 (concurrent with each
                # other) into one shared PSUM bank.
                if jt == 0:
                    pav_tiles[dt] = ps.tile([P, M], F32, tag="pav", bufs=1, name="pav")
                    sums_tiles[dt] = ps.tile(
                        [33, M], F32, tag="sums", bufs=1, name="sums"
                    )
                aT = aT_tiles[dt]
                pav = pav_tiles[dt]
                sums = sums_tiles[dt]
                st, sp = (jt == 0), (jt == NT - 1)
                slot0 = (jt * DT + dt) * 2
                rhs0 = aT[:, (jt * 2) * M : (jt * 2 + 1) * M]
                rhs1 = aT[:, (jt * 2 + 1) * M : (jt * 2 + 2) * M]
                nc.tensor.matmul(
                    pav[0:64, :], lhsT=vA[:, slot0, :], rhs=rhs0,
                    start=st, stop=sp, skip_group_check=True,
                )
                nc.tensor.matmul(
                    pav[64:128, :], lhsT=vA[:, slot0 + 1, :], rhs=rhs1,
                    start=st, stop=sp, tile_position=(0, 64), skip_group_check=True,
                )
                nc.tensor.matmul(
                    sums[0:1, :], lhsT=ones_sb[:, 0:1], rhs=rhs0,
                    start=st, stop=sp, skip_group_check=True,
                )
                nc.tensor.matmul(
                    sums[32:33, :], lhsT=ones_sb[:, 0:1], rhs=rhs1,
                    start=st, stop=sp, tile_position=(0, 32), skip_group_check=True,
                )

            def emit_norm_gate(dt):
                pav = pav_tiles.pop(dt)
                sums = sums_tiles.pop(dt)
                # stage sum rows into SBUF for the broadcast MMs
                nc.vector.tensor_copy(out=srow[0:1, 0, :], in_=sums[0:1, :])
                nc.vector.tensor_copy(out=srow[32:33, 1, :], in_=sums[32:33, :])
                prf = ps.tile([P, M], F32, tag="small", bufs=2, name="prf")
                nc.tensor.matmul(
                    prf[0:64, :],
                    lhsT=ones_sb[0:1, 0:64],
                    rhs=srow[0:1, 0, :],
                    start=True,
                    stop=True,
                    skip_group_check=True,
                )
                nc.tensor.matmul(
                    prf[64:128, :],
                    lhsT=ones_sb[32:33, 0:64],
                    rhs=srow[32:33, 1, :],
                    start=True,
                    stop=True,
                    tile_position=(32, 64),
                    skip_group_check=True,
                )
                # 1/s via exp(-ln(s)) on ACT (Reciprocal table is banned)
                grec = rings.tile([P, M], F32, tag="grec", bufs=2, name="grec")
                nc.scalar.activation(out=grec, in_=prf, func=Ln, scale=1.0)
                nc.scalar.activation(out=grec, in_=grec, func=Exp, scale=-1.0)
                geff = rings.tile([P, M], F32, tag="geff", bufs=2, name="geff")
                nc.vector.tensor_tensor(geff, grec, gT[:, dt, :], MUL)
                nc.vector.tensor_tensor(
                    gatedT[0:64, dt, :], pav[0:64, :], geff[0:64, :], MUL
                )
                nc.vector.tensor_tensor(
                    gatedT[64:128, dt, :], pav[64:128, :], geff[64:128, :], MUL
                )

            # ---------------- main pipeline ----------------
            # own k/v first (feeds the exchange), then q/g fill the gap
            # while the AllGather is in flight.
            for dt in range(DT):
                emit_kown(dt)
            for jt in range(JO):
                emit_vown(jt)
            emit_exchange()
            # partial q/g before attention (covers the AllGather latency);
            # the rest feeds the attention loop as PE filler.
            emit_qT(0)
            emit_qT(1)
            emit_qT(2)
            emit_g(0)
            emit_g(1)

            fill = {
                0: [lambda: emit_qT(3), lambda: emit_g(2), lambda: emit_g(3)],
                1: [],
                2: [],
                3: [],
            }

            # attention: AV trails QK by 2 j-tiles within the same dt;
            # norm_gate of dt-1 is deferred into dt's loop (jt==1) to keep
            # its ACT recip off the exp critical path.
            for dt in range(DT):
                aT_tiles[dt] = rings.tile(
                    [P, NT * 2 * M], BF16, tag="aT", bufs=2, name="aT"
                )
                work = list(fill[dt])
                for jt in range(NT):
                    emit_qk(dt, jt)
                    if jt == 1 and dt > 0:
                        emit_norm_gate(dt - 1)
                    if jt >= 2:
                        emit_av(dt, jt - 2)
                    n_pop = (len(work) + NT - 1 - jt) // (NT - jt)
                    for _ in range(n_pop):
                        work.pop(0)()
                emit_av(dt, NT - 2)
                emit_av(dt, NT - 1)
            emit_norm_gate(DT - 1)

            # ---------------- output projection ----------------
            for ib in range(IB):
                po = ps.tile([P, 2, 512], F32, tag="pd", bufs=2, name="po")
                for dt in range(DT):
                    for dh in range(2):
                        nc.tensor.matmul(
                            po[:, dh, :],
                            lhsT=gatedT[:, dt, ib * P : (ib + 1) * P],
                            rhs=wo_sb[:, dt, dh * 512 : (dh + 1) * 512],
                            start=(dt == 0),
                            stop=(dt == DT - 1),
                            skip_group_check=True,
                        )
                osb = rings.tile([P, D], F32, tag="osb", bufs=2, name="osb")
                nc.vector.tensor_tensor(
                    osb, po.rearrange("p a b -> p (a b)"), bob_sb, ADD
                )
                nc.sync.dma_start(
                    out=out_ext.rearrange("(ib p) d -> p ib d", p=P)[:, ib, :],
                    in_=osb,
                )

    _legalize_waits(nc)
    return nc


_NC_CACHE = None


def _get_graph():
    global _NC_CACHE
    if _NC_CACHE is None:
        _NC_CACHE = _build_graph()
    return _NC_CACHE


def _prepare_in_maps(x, mask, attn_bias, Wq, Wkv, Wg, bg, Wo, bo):
    x = np.asarray(x, dtype=np.float32)
    mask = np.asarray(mask, dtype=bool)
    attn_bias = np.asarray(attn_bias, dtype=np.float32)
    Wq = np.asarray(Wq, dtype=np.float32)
    Wkv = np.asarray(Wkv, dtype=np.float32)
    Wg = np.asarray(Wg, dtype=np.float32)
    bg = np.asarray(bg, dtype=np.float32)
    Wo = np.asarray(Wo, dtype=np.float32)
    bo = np.asarray(bo, dtype=np.float32)

    wq_scaled = np.ascontiguousarray(Wq * np.float32(DH**-0.5)).astype(
        ml_dtypes.bfloat16
    )
    nbg = np.ascontiguousarray(bg.reshape(DT, P).T)
    wkv_b = Wkv.astype(ml_dtypes.bfloat16)
    wg_b = Wg.astype(ml_dtypes.bfloat16)
    wo_b = Wo.astype(ml_dtypes.bfloat16)
    bob = np.ascontiguousarray(np.broadcast_to(bo.reshape(1, D), (P, D)))

    # Fold the attention mask into the bias (both sides), then exponentiate:
    # the kernel computes attn = exp(qk) * exp(bias).  Masked entries -> 0.
    m2 = mask[:, None, :, None] & mask[:, None, None, :]  # (B, 1, n, n)
    bias_eff = np.where(m2, attn_bias, np.float32(-np.inf))
    bias_eff = np.exp(bias_eff)

    in_maps = []
    for c in range(N_CORES):
        b, r = divmod(c, 2)
        xq = np.ascontiguousarray(x[b, r * M : (r + 1) * M]).astype(ml_dtypes.bfloat16)
        # bias slice: own query rows, transposed to [dt, j, h(2), i]
        bc = bias_eff[b][:, r * M : (r + 1) * M, :]  # (H, M, N)
        bcT = bc.reshape(DT, 2, M, N).transpose(0, 3, 1, 2)  # (DT, N, 2, M)
        in_maps.append(
            {
                "x": xq,
                "bias": np.ascontiguousarray(bcT).astype(ml_dtypes.bfloat16),
                "wq": wq_scaled,
                "wkv": wkv_b,
                "wg": wg_b,
                "nbg": nbg,
                "wo": wo_b,
                "bob": bob,
            }
        )
    return in_maps


def _assemble(results):
    out = np.empty((B, N, D), dtype=np.float32)
    for c in range(N_CORES):
        b, r = divmod(c, 2)
        out[b, r * M : (r + 1) * M, :] = results[c]["out"]
    return out


def _run(in_maps, trace=False):
    nc = _get_graph()
    last_err = None
    for attempt in range(3):
        try:
            return run_bass_kernel_spmd(
                nc, in_maps, core_ids=list(range(N_CORES)), trace=trace
            )
        except Exception as e:  # transient device faults recover on retry
            last_err = e
    raise last_err


def kernel(**inputs):
    in_maps = _prepare_in_maps(**inputs)
    res = _run(in_maps)
    return _assemble(res.results)


def kernel_traced(**inputs):
    """Like kernel() but with NTFF profiling; returns (out, exec_time_ns)."""
    in_maps = _prepare_in_maps(**inputs)
    res = _run(in_maps, trace=True)
    return _assemble(res.results), res.exec_time_ns


# revision 36
# speedup vs baseline: 1.0037x; 1.0037x over previous
"""Self-contained Trainium2 Bass kernel for gated attention (sparse_attention).

Reference computation (per batch b):
    q = split_heads(x @ Wq) * DH**-0.5        # (H, n, DH)
    k, v = split_heads(x @ Wkv)               # (H, n, DH) each
    dots = q k^T + attn_bias ; masked softmax over j
    out = (attn @ v) reshaped to (n, H*DH)
    out = out * sigmoid(x @ Wg + bg)
    return out @ Wo + bo

Sharding: 8 cores = 4 batches x 2 query-row halves, physical j
coordinates.  Each core projects k/v only for its OWN 512 rows; the
halves are exchanged between pair cores with a 2-rank AllGather
(DRAM bounce), so k/v work is not duplicated.  Per-core outputs are
disjoint.

v3 design: one fully-pipelined TileContext (no phase barriers).
  - x input is just the core's own 512 rows; x^T built with 32 PE
    transposes.
  - bias (host-side exp'ed, bf16) streams from t=0 while projections
    and the k/v exchange run.
  - attention per head-pair dt: QK pair (row-groups) and AV pair
    (col-groups) run as concurrent matmul pairs; row sums are two
    concurrent M=1 matmuls into one PSUM bank; 1/sum comes from
    exp(-ln(s)) on ACT applied to the PE-broadcast tile.
  - engine split: ACT = exp + sigmoid + recip; DVE = PSUM drains,
    gating, output adds, ~60% of bias mults; GPSIMD = the rest.
"""
import sys
import types

import numpy as np
import ml_dtypes

# ---------------------------------------------------------------------------
# Environment shims (axon container): NTFF profile hook + walrus drain fix.
# ---------------------------------------------------------------------------


def _install_axon_ntff_hook():
    try:
        import antenv
    except ImportError:
        return
    if hasattr(antenv, "axon_hooks"):
        return
    mod = types.ModuleType("antenv.axon_hooks")
    mod._hook = None

    def set_axon_ntff_profile_hook(h):
        mod._hook = h

    def get_axon_ntff_profile_hook():
        return mod._hook

    mod.set_axon_ntff_profile_hook = set_axon_ntff_profile_hook
    mod.get_axon_ntff_profile_hook = get_axon_ntff_profile_hook
    sys.modules["antenv.axon_hooks"] = mod
    antenv.axon_hooks = mod
    try:
        from trn_agent_boot.trn_boot import _ntff_profile_via_ctypes

        hook = _ntff_profile_via_ctypes("/opt/axon/libaxon_pjrt.so")
        if hook is not None:
            set_axon_ntff_profile_hook(hook)
    except Exception:
        pass


_install_axon_ntff_hook()

import concourse.bass as bass  # noqa: E402
import concourse.tile as tile  # noqa: E402
import concourse.mybir as mybir  # noqa: E402
from concourse.bass_utils import run_bass_kernel_spmd  # noqa: E402
from concourse.masks import make_identity  # noqa: E402
from concourse.tile import ScopedClock  # noqa: E402


def _patch_tile_drain():
    """The installed walrus accepts only one sync-wait per Drain; Tile's
    tail drain carries one wait per outstanding semaphore.  Split them
    across a chain of single-wait drains (same engine => same semantics)."""

    def _drain_and_barrier(self, tick_clock, wait_clock):
        nc = self.nc
        drain_inst = nc.sync.drain()
        wait_clock.add_sem_waits(
            drain_inst.ins, ScopedClock({None: tick_clock.global_clock})
        )
        si = drain_inst.ins.sync_info
        if si is not None and len(si.on_wait) > 1:
            waits = list(si.on_wait)
            drain_inst.ins.sync_info = mybir.SyncInfo(
                on_wait=waits[:1], on_update=list(si.on_update)
            )
            for w in waits[1:]:
                extra = nc.sync.drain()
                extra.ins.sync_info = mybir.SyncInfo(on_wait=[w], on_update=[])

        nc.all_engine_barrier()
        assert self.sems is not None
        popped = nc._tile_sem_poison_stack.pop()
        assert popped is self._sem_poison
        nc.clear_and_free_semaphores(list(self.sems.allocated().values()))
        nc.all_engine_barrier()

    tile.TileContext._drain_and_barrier = _drain_and_barrier


_patch_tile_drain()


def _legalize_waits(nc, max_waits=1):
    """Walrus in this container accepts at most one sync-wait per lowered
    instruction.  Move surplus waits onto single-wait NoOps inserted just
    before the instruction on the same engine."""
    nid = 0
    n_split = 0
    for f in nc.m.functions:
        for bb in f.blocks:
            out = []
            changed = False
            for inst in bb.instructions:
                si = inst.sync_info
                if si is not None and len(si.on_wait) > max_waits:
                    waits = list(si.on_wait)
                    for w in waits[:-1]:
                        nop = mybir.InstNoOp(name=f"WSPLIT-{nid}")
                        nid += 1
                        nop.engine = inst.engine
                        nop.sync_info = mybir.SyncInfo(on_wait=[w], on_update=[])
                        out.append(nop)
                    inst.sync_info = mybir.SyncInfo(
                        on_wait=[waits[-1]], on_update=list(si.on_update)
                    )
                    changed = True
                    n_split += 1
                out.append(inst)
            if changed:
                bb.instructions = out
    return n_split


# ---------------------------------------------------------------------------
# Problem constants (hardcoded per spec).
# ---------------------------------------------------------------------------
B, N, D = 4, 1024, 1024
H, DH = 8, 64
INNER = H * DH  # 512
M = N // 2  # 512 query rows per core
N_CORES = 8
P = 128
F32 = mybir.dt.float32
BF16 = mybir.dt.bfloat16

CT = D // P  # 8 contraction tiles over feature dim
DT = INNER // P  # 4 head pairs
NT = N // P  # 8 tiles over keys j (full)
JO = M // P  # 4 own j tiles
IB = M // P  # 4 tiles over query rows
KVW = DT * M + JO * M  # 4096: packed exchange row width (k | v)

Exp = mybir.ActivationFunctionType.Exp
Ln = mybir.ActivationFunctionType.Ln
Sigmoid = mybir.ActivationFunctionType.Sigmoid
MUL = mybir.AluOpType.mult
ADD = mybir.AluOpType.add


def _build_graph():
    nc = bass.Bass()
    x_ext = nc.declare_dram_parameter("x", [M, D], BF16, isOutput=False)
    bias_ext = nc.declare_dram_parameter("bias", [DT, N, 2, M], BF16, isOutput=False)
    wq_ext = nc.declare_dram_parameter("wq", [D, INNER], BF16, isOutput=False)
    wkv_ext = nc.declare_dram_parameter("wkv", [D, 2 * INNER], BF16, isOutput=False)
    wg_ext = nc.declare_dram_parameter("wg", [D, INNER], BF16, isOutput=False)
    nbg_ext = nc.declare_dram_parameter("nbg", [P, DT], F32, isOutput=False)
    wo_ext = nc.declare_dram_parameter("wo", [INNER, D], BF16, isOutput=False)
    bob_ext = nc.declare_dram_parameter("bob", [P, D], F32, isOutput=False)
    out_ext = nc.declare_dram_parameter("out", [M, D], F32, isOutput=True)

    with tile.TileContext(nc) as tc:
        with (
            tc.tile_pool(name="persist", bufs=1) as persist,
            tc.tile_pool(name="rings", bufs=1) as rings,
            tc.tile_pool(name="dram", bufs=1, space="DRAM") as dram,
            tc.tile_pool(name="ps", bufs=1, space="PSUM") as ps,
        ):
            # ---------------- persistent SBUF ----------------
            x_sb = persist.tile([P, IB, D], BF16, name="x_sb")
            xT = persist.tile([P, CT, M], BF16, name="xT")  # own x^T [c, i]
            kT = persist.tile([P, DT, N], BF16, name="kT")  # k^T  [dI, j] full
            vA = persist.tile([P, DT * NT * 2, 64], BF16, name="vA")  # v slots
            kvx = persist.tile([P, KVW], BF16, name="kvx")  # exchange staging
            qT = persist.tile([P, DT, M], BF16, name="qT")
            gT = persist.tile([P, DT, M], BF16, name="gT")  # sigmoid gates
            gatedT = persist.tile([P, DT, M], BF16, name="gatedT")
            wq_sb = persist.tile([P, CT, INNER], BF16, name="wq_sb")
            wkv_sb = persist.tile([P, CT, 2 * INNER], BF16, name="wkv_sb")
            wg_sb = persist.tile([P, CT, INNER], BF16, name="wg_sb")
            wo_sb = persist.tile([P, DT, D], BF16, name="wo_sb")
            nbg_sb = persist.tile([P, DT], F32, name="nbg_sb")
            bob_sb = persist.tile([P, D], F32, name="bob_sb")
            srow = persist.tile([P, 2, M], BF16, name="srow")  # sums @p0/p32
            ones_sb = persist.tile([P, P], BF16, name="ones_sb")
            ident = persist.tile([P, P], BF16, name="ident")
            warm_src = persist.tile([P, 512], BF16, name="warm_src")

            bias_tiles = {
                dt: rings.tile([P, NT * 2 * M], BF16, tag="bias", bufs=3, name="bias_sb")
                for dt in range(DT)
            }

            # DRAM bounce buffers for the pairwise k/v AllGathers
            kb_in = dram.tile([P, DT * M], BF16, name="kb_in")
            kb_out = dram.tile([2, P, DT * M], BF16, name="kb_out")
            vb_in = dram.tile([P, JO * INNER], BF16, name="vb_in")
            vb_out = dram.tile([2, P, JO * INNER], BF16, name="vb_out")

            # ---------------- DMA: sync ring ----------------
            nc.sync.dma_start(
                out=x_sb, in_=x_ext.rearrange("(nt p) d -> p nt d", p=P)
            )
            PAIRS = [[0, 1], [2, 3], [4, 5], [6, 7]]

            def emit_k_exchange():
                # Emitted AFTER the k projections (Tile deps are
                # program-order; an earlier read would see garbage).
                nc.sync.dma_start(out=kb_in, in_=kvx[:, 0 : DT * M])
                nc.gpsimd.collective_compute(
                    "AllGather",
                    mybir.AluOpType.bypass,
                    ins=[kb_in.opt()],
                    outs=[kb_out.opt()],
                    replica_groups=PAIRS,
                )
                for t in range(2):
                    nc.sync.dma_start(
                        out=kT[:, :, t * M : (t + 1) * M],
                        in_=kb_out[t].rearrange("p (dt i) -> p dt i", dt=DT),
                    )

            def emit_v_exchange():
                nc.sync.dma_start(out=vb_in, in_=kvx[:, DT * M :])
                nc.gpsimd.collective_compute(
                    "AllGather",
                    mybir.AluOpType.bypass,
                    ins=[vb_in.opt()],
                    outs=[vb_out.opt()],
                    replica_groups=PAIRS,
                )
                # vA slot order is (jt_phys, dt, h) so each half is one
                # contiguous DMA
                for t in range(2):
                    nc.sync.dma_start(
                        out=vA[:, t * 32 : (t + 1) * 32, :], in_=vb_out[t]
                    )
                for dt in range(2, DT):
                    nc.sync.dma_start(
                        out=bias_tiles[dt].rearrange(
                            "p (jt h i) -> p jt h i", jt=NT, h=2
                        ),
                        in_=bias_ext[dt].rearrange("(jt p) h i -> p jt h i", p=P),
                    )

            # ---------------- DMA: scalar ring = weights, then bias 0/1 ----
            nc.scalar.dma_start(
                out=wkv_sb, in_=wkv_ext.rearrange("(ct p) i -> p ct i", p=P)
            )
            nc.scalar.dma_start(
                out=wq_sb, in_=wq_ext.rearrange("(ct p) i -> p ct i", p=P)
            )
            nc.scalar.dma_start(
                out=wg_sb, in_=wg_ext.rearrange("(ct p) i -> p ct i", p=P)
            )
            nc.scalar.dma_start(out=nbg_sb, in_=nbg_ext[:])
            for dt in range(2):
                nc.scalar.dma_start(
                    out=bias_tiles[dt].rearrange(
                        "p (jt h i) -> p jt h i", jt=NT, h=2
                    ),
                    in_=bias_ext[dt].rearrange("(jt p) h i -> p jt h i", p=P),
                )
            nc.scalar.dma_start(
                out=wo_sb, in_=wo_ext.rearrange("(dt p) i -> p dt i", p=P)
            )
            nc.scalar.dma_start(out=bob_sb, in_=bob_ext[:])

            # ---------------- init ----------------
            nc.vector.memset(warm_src, 1.0)
            nc.vector.memset(ones_sb, 1.0)
            make_identity(nc, ident)

            # ---------------- PE warmup ----------------
            for i in range(6):
                wt = ps.tile([P, 512], F32, tag="small", bufs=2, name="warm")
                nc.tensor.matmul(
                    wt, lhsT=warm_src[:, 0:P], rhs=warm_src, start=True, stop=True,
                    skip_group_check=True,
                )

            # ---------------- x^T via PE transposes ----------------
            for nt in range(IB):
                for ct in range(CT):
                    pt = ps.tile([P, P], BF16, tag="small", bufs=2, name="pt")
                    nc.tensor.transpose(
                        pt, x_sb[:, nt, ct * P : (ct + 1) * P], ident
                    )
                    nc.vector.tensor_copy(
                        out=xT[:, ct, nt * P : (nt + 1) * P], in_=pt
                    )

            # ---------------- helpers ----------------
            def emit_kown(dt):
                # k^T own half -> kvx[:, dt*M:(dt+1)*M]
                pt = ps.tile([P, M], F32, tag="small", bufs=2, name="pk")
                for ct in range(CT):
                    nc.tensor.matmul(
                        pt,
                        lhsT=wkv_sb[:, ct, dt * P : (dt + 1) * P],
                        rhs=xT[:, ct, :],
                        start=(ct == 0),
                        stop=(ct == CT - 1),
                    )
                nc.vector.tensor_copy(out=kvx[:, dt * M : (dt + 1) * M], in_=pt)

            def emit_vown(jt):
                # v rows for own j-tile jt (all heads) -> kvx v section
                pt = ps.tile([P, INNER], F32, tag="small", bufs=2, name="pv")
                for ct in range(CT):
                    nc.tensor.matmul(
                        pt,
                        lhsT=xT[:, ct, jt * P : (jt + 1) * P],
                        rhs=wkv_sb[:, ct, INNER:],
                        start=(ct == 0),
                        stop=(ct == CT - 1),
                    )
                nc.vector.tensor_copy(
                    out=kvx[:, DT * M + jt * INNER : DT * M + (jt + 1) * INNER],
                    in_=pt,
                )

            def emit_qT(dt):
                pt = ps.tile([P, M], F32, tag="small", bufs=2, name="pq")
                for ct in range(CT):
                    nc.tensor.matmul(
                        pt,
                        lhsT=wq_sb[:, ct, dt * P : (dt + 1) * P],
                        rhs=xT[:, ct, :],
                        start=(ct == 0),
                        stop=(ct == CT - 1),
                    )
                nc.vector.tensor_copy(out=qT[:, dt, :], in_=pt)

            def emit_g(dt):
                pt = ps.tile([P, M], F32, tag="small", bufs=2, name="pg")
                for ct in range(CT):
                    nc.tensor.matmul(
                        pt,
                        lhsT=wg_sb[:, ct, dt * P : (dt + 1) * P],
                        rhs=xT[:, ct, :],
                        start=(ct == 0),
                        stop=(ct == CT - 1),
                    )
                nc.scalar.activation(
                    out=gT[:, dt, :],
                    in_=pt,
                    func=Sigmoid,
                    scale=1.0,
                    bias=nbg_sb[:, dt : dt + 1],
                )

            # attention state
            aT_tiles = {}
            pav_tiles = {}
            sums_tiles = {}

            def emit_qk(dt, jt):
                pd = ps.tile([P, 2, M], F32, tag="pd", bufs=2, name="pd")
                for hi in range(2):
                    po = 64 * hi
                    nc.tensor.matmul(
                        pd[:, hi, :],
                        lhsT=kT[po : po + 64, dt, jt * P : (jt + 1) * P],
                        rhs=qT[po : po + 64, dt, :],
                        start=True,
                        stop=True,
                    )
                aT = aT_tiles[dt]
                dst = aT[:, jt * 2 * M : (jt + 1) * 2 * M]
                nc.scalar.activation(
                    out=dst, in_=pd.rearrange("p a b -> p (a b)"), func=Exp, scale=1.0
                )
                beng = nc.gpsimd if jt in (1, 3, 5) else nc.vector
                beng.tensor_tensor(
                    dst, dst, bias_tiles[dt][:, jt * 2 * M : (jt + 1) * 2 * M], MUL
                )

            def emit_av(dt, jt):
                # AV pair on disjoint col groups (concurrent), then the two
                # M=1 row-sum matmuls on groups 0/1 (concurrent with each
                # other) into one shared PSUM bank.
                if jt == 0:
                    pav_tiles[dt] = ps.tile([P, M], F32, tag="pav", bufs=1, name="pav")
                    sums_tiles[dt] = ps.tile(
                        [33, M], F32, tag="sums", bufs=1, name="sums"
                    )
                aT = aT_tiles[dt]
                pav = pav_tiles[dt]
                sums = sums_tiles[dt]
                st, sp = (jt == 0), (jt == NT - 1)
                slot0 = (jt * DT + dt) * 2
                rhs0 = aT[:, (jt * 2) * M : (jt * 2 + 1) * M]
                rhs1 = aT[:, (jt * 2 + 1) * M : (jt * 2 + 2) * M]
                nc.tensor.matmul(
                    pav[0:64, :], lhsT=vA[:, slot0, :], rhs=rhs0,
                    start=st, stop=sp, skip_group_check=True,
                )
                nc.tensor.matmul(
                    pav[64:128, :], lhsT=vA[:, slot0 + 1, :], rhs=rhs1,
                    start=st, stop=sp, tile_position=(0, 64), skip_group_check=True,
                )
                nc.tensor.matmul(
                    sums[0:1, :], lhsT=ones_sb[:, 0:1], rhs=rhs0,
                    start=st, stop=sp, skip_group_check=True,
                )
                nc.tensor.matmul(
                    sums[32:33, :], lhsT=ones_sb[:, 0:1], rhs=rhs1,
                    start=st, stop=sp, tile_position=(0, 32), skip_group_check=True,
                )

            def emit_norm_gate(dt):
                pav = pav_tiles.pop(dt)
                sums = sums_tiles.pop(dt)
                # stage sum rows into SBUF for the broadcast MMs
                nc.vector.tensor_copy(out=srow[0:1, 0, :], in_=sums[0:1, :])
                nc.vector.tensor_copy(out=srow[32:33, 1, :], in_=sums[32:33, :])
                prf = ps.tile([P, M], F32, tag="small", bufs=2, name="prf")
                nc.tensor.matmul(
                    prf[0:64, :],
                    lhsT=ones_sb[0:1, 0:64],
                    rhs=srow[0:1, 0, :],
                    start=True,
                    stop=True,
                    skip_group_check=True,
                )
                nc.tensor.matmul(
                    prf[64:128, :],
                    lhsT=ones_sb[32:33, 0:64],
                    rhs=srow[32:33, 1, :],
                    start=True,
                    stop=True,
                    tile_position=(32, 64),
                    skip_group_check=True,
                )
                # 1/s via exp(-ln(s)) on ACT (Reciprocal table is banned)
                grec = rings.tile([P, M], F32, tag="grec", bufs=2, name="grec")
                nc.scalar.activation(out=grec, in_=prf, func=Ln, scale=1.0)
                nc.scalar.activation(out=grec, in_=grec, func=Exp, scale=-1.0)
                geff = rings.tile([P, M], F32, tag="geff", bufs=2, name="geff")
                nc.vector.tensor_tensor(geff, grec, gT[:, dt, :], MUL)
                nc.vector.tensor_tensor(
                    gatedT[0:64, dt, :], pav[0:64, :], geff[0:64, :], MUL
                )
                nc.vector.tensor_tensor(
                    gatedT[64:128, dt, :], pav[64:128, :], geff[64:128, :], MUL
                )

            # ---------------- main pipeline ----------------
            # own k/v first (feeds the exchange), then q/g fill the gap
            # while the AllGather is in flight.
            for dt in range(DT):
                emit_kown(dt)
            emit_k_exchange()
            for jt in range(JO):
                emit_vown(jt)
            emit_v_exchange()
            # q/g projections fill the PE while the AllGathers are in flight
            for dt in range(DT):
                emit_qT(dt)
            for dt in range(DT):
                emit_g(dt)

            # attention: AV trails QK by 2 j-tiles within the same dt
            for dt in range(DT):
                aT_tiles[dt] = rings.tile(
                    [P, NT * 2 * M], BF16, tag="aT", bufs=2, name="aT"
                )
                for jt in range(NT):
                    emit_qk(dt, jt)
                    if jt >= 2:
                        emit_av(dt, jt - 2)
                emit_av(dt, NT - 2)
                emit_av(dt, NT - 1)
                emit_norm_gate(dt)

            # ---------------- output projection ----------------
            for ib in range(IB):
                po = ps.tile([P, 2, 512], F32, tag="pd", bufs=2, name="po")
                for dt in range(DT):
                    for dh in range(2):
                        nc.tensor.matmul(
                            po[:, dh, :],
                            lhsT=gatedT[:, dt, ib * P : (ib + 1) * P],
                            rhs=wo_sb[:, dt, dh * 512 : (dh + 1) * 512],
                            start=(dt == 0),
                            stop=(dt == DT - 1),
                            skip_group_check=True,
                        )
                osb = rings.tile([P, D], F32, tag="osb", bufs=2, name="osb")
                nc.vector.tensor_tensor(
                    osb, po.rearrange("p a b -> p (a b)"), bob_sb, ADD
                )
                nc.sync.dma_start(
                    out=out_ext.rearrange("(ib p) d -> p ib d", p=P)[:, ib, :],
                    in_=osb,
                )

    _legalize_waits(nc)
    return nc


_NC_CACHE = None


def _get_graph():
    global _NC_CACHE
    if _NC_CACHE is None:
        _NC_CACHE = _build_graph()
    return _NC_CACHE


def _prepare_in_maps(x, mask, attn_bias, Wq, Wkv, Wg, bg, Wo, bo):
    x = np.asarray(x, dtype=np.float32)
    mask = np.asarray(mask, dtype=bool)
    attn_bias = np.asarray(attn_bias, dtype=np.float32)
    Wq = np.asarray(Wq, dtype=np.float32)
    Wkv = np.asarray(Wkv, dtype=np.float32)
    Wg = np.asarray(Wg, dtype=np.float32)
    bg = np.asarray(bg, dtype=np.float32)
    Wo = np.asarray(Wo, dtype=np.float32)
    bo = np.asarray(bo, dtype=np.float32)

    wq_scaled = np.ascontiguousarray(Wq * np.float32(DH**-0.5)).astype(
        ml_dtypes.bfloat16
    )
    nbg = np.ascontiguousarray(bg.reshape(DT, P).T)
    wkv_b = Wkv.astype(ml_dtypes.bfloat16)
    wg_b = Wg.astype(ml_dtypes.bfloat16)
    wo_b = Wo.astype(ml_dtypes.bfloat16)
    bob = np.ascontiguousarray(np.broadcast_to(bo.reshape(1, D), (P, D)))

    # Fold the attention mask into the bias (both sides), then exponentiate:
    # the kernel computes attn = exp(qk) * exp(bias).  Masked entries -> 0.
    m2 = mask[:, None, :, None] & mask[:, None, None, :]  # (B, 1, n, n)
    bias_eff = np.where(m2, attn_bias, np.float32(-np.inf))
    bias_eff = np.exp(bias_eff)

    in_maps = []
    for c in range(N_CORES):
        b, r = divmod(c, 2)
        xq = np.ascontiguousarray(x[b, r * M : (r + 1) * M]).astype(ml_dtypes.bfloat16)
        # bias slice: own query rows, transposed to [dt, j, h(2), i]
        bc = bias_eff[b][:, r * M : (r + 1) * M, :]  # (H, M, N)
        bcT = bc.reshape(DT, 2, M, N).transpose(0, 3, 1, 2)  # (DT, N, 2, M)
        in_maps.append(
            {
                "x": xq,
                "bias": np.ascontiguousarray(bcT).astype(ml_dtypes.bfloat16),
                "wq": wq_scaled,
                "wkv": wkv_b,
                "wg": wg_b,
                "nbg": nbg,
                "wo": wo_b,
                "bob": bob,
            }
        )
    return in_maps


def _assemble(results):
    out = np.empty((B, N, D), dtype=np.float32)
    for c in range(N_CORES):
        b, r = divmod(c, 2)
        out[b, r * M : (r + 1) * M, :] = results[c]["out"]
    return out


def _run(in_maps, trace=False):
    nc = _get_graph()
    last_err = None
    for attempt in range(3):
        try:
            return run_bass_kernel_spmd(
                nc, in_maps, core_ids=list(range(N_CORES)), trace=trace
            )
        except Exception as e:  # transient device faults recover on retry
            last_err = e
    raise last_err


def kernel(**inputs):
    in_maps = _prepare_in_maps(**inputs)
    res = _run(in_maps)
    return _assemble(res.results)


def kernel_traced(**inputs):
    """Like kernel() but with NTFF profiling; returns (out, exec_time_ns)."""
    in_maps = _prepare_in_maps(**inputs)
    res = _run(in_maps, trace=True)
    return _assemble(res.results), res.exec_time_ns
